# revision 2
# baseline (speedup 1.0000x reference)
"""Trainium2 Bass kernel v2 for nn_AttentionTIE (TIE-style edge-LayerNorm attention).

Sharding: 8 cores = (batch b = core//2) x (receiver-row half = core%2).

v2 redesign vs v1 (cost-model: ACT 139us / DVE 136us / PE 116us busy, 188us wall):
  - softmax exp moved OFF the ACT engine onto the DVE via the Schraudolph
    bit trick: P = bitcast_bf16(int16(A*uc + B - delta)), A = 128/ln2.
    One 4x-mode tensor_scalar (max-clamp + bias add) replaces the ACT Exp
    pass + accumulator read. ACT keeps only Ln+Exp for T = A/sqrt(std2)
    (Rsqrt tables are banned in bass; Ln/Exp share table set 6).
  - whole score/std2 pipeline computed TRANSPOSED (j on partitions,
    "B layout"): w_var rides the Ln bias (per-partition), u_eps and alpha
    enter as rank-1 matmuls, the mask (host-prepped fp8 = -60*mask^T) via
    an identity matmul. P^T comes out of the exp directly - no PE
    transposes of P, and PT^T = P^T * tccT is ONE 2x tensor_tensor.
  - denominator = sum_j P via 1-wide PE matmuls (P^T lhsT @ ones): output
    width 1 => ~1 cycle each on the cost model, exact f32 PSUM accumulation.
  - uc kept in fp16 (not bf16): halves the exp-argument quantization error
    at identical DVE cost (4x mode needs any 2-byte dtype).
  - epilogue reciprocal via reciprocal_approx_fast (v1 lost 8.5us to
    bit-exact InstReciprocal on [128,1] columns).
  - phase 1: own-half v_s reuse (v1 recomputed it from re-streamed inputs),
    single-matmul m_s, row-ops spread over Pool/ACT.

Layout summary (all tiles [partition, free]):
  v_sT=[C,N] f32r (centered in place); v_s_aug=[j,(C+2)*24] bf16 (v_s|1|m_s)
  per (ic: 3 x 512 receiver cols, jt: 24 x 128 sender rows):
    ps_v[j,i] = c_sT[jt]^T vr_s + 1 (x) u_eps        (u via rank-1)
    ps_s[j,i] = c_sT[jt]^T qT + maskT + 1 (x) alpha  (mask fp8 idb-matmul)
    tccT = Exp(-.5*Ln(ps_v + w_col) + lnA)  -> fp16  (= A/std)
    uc   = ps_s * tccT                      -> fp16  (DVE tt)
    P^T  = int16(max(uc,-13000) + B-delta)  -> bf16 bits (DVE ts, 4x)
    PTT  = P^T * tccT                       -> bf16  (DVE tt, 2x)
    pv[i,0:C+2] += PTT^T @ v_s_aug[jt]; den[i] += P^T @ 1
  epilogue: out = (pv + A_i v_r - (m_r A_i + MS)) / (A*den); three
  projections as in v1.
"""
import sys

sys.path.insert(0, "/opt/trn_rl_repo")

import numpy as np
import ml_dtypes  # noqa: E402
from contextlib import ExitStack  # noqa: E402

import concourse.bass as bass  # noqa: E402
import concourse.tile as tile  # noqa: E402
from concourse import bacc  # noqa: E402
from concourse import mybir  # noqa: E402
from concourse.bass_utils import run_bass_kernel_spmd  # noqa: E402

N, B, C = 3072, 4, 128
NO = N // 2          # own receiver rows per core
ITI = NO // 128      # 12 i-tiles
ICH = NO // 512      # 3 i-chunks (512)
JT = N // 128        # 24 j-tiles
JCH = N // 512       # 6 sender chunks
EPS = 1e-5
SCALE = C ** -0.5
MASKM = 60.0
AEXP = 128.0 / float(np.log(2.0))     # 184.6649652
LNA = float(np.log(AEXP))             # 5.21868...
BEXP = 127.0 * 128.0                  # 16256
DELTA = 5.1                           # floor-calibrated Schraudolph bias
UCCLAMP = -13000.0

F32 = mybir.dt.float32
F32R = mybir.dt.float32r
BF16 = mybir.dt.bfloat16
FP16 = mybir.dt.float16
FP8 = mybir.dt.float8e4
I16 = mybir.dt.int16
AF = mybir.ActivationFunctionType
ALU = mybir.AluOpType
AX = mybir.AxisListType

_CACHE = {}


def _build_program():
    """One program for all cores: the host permutes the sender axis so each
    core's own receiver half is always sender columns [0, NO)."""
    nc = bacc.Bacc("TRN2", target_bir_lowering=False, debug=False, num_devices=8)

    def din(name, shape, dtype=F32):
        return nc.dram_tensor(name, list(shape), dtype, kind="ExternalInput").ap()

    def dout(name, shape, dtype=F32):
        return nc.dram_tensor(name, list(shape), dtype, kind="ExternalOutput").ap()

    xT_d = din("xT", [C, N])
    sendT_d = din("sendT", [C, N])
    res_sT_d = din("res_sT", [C, N])
    recvTo_d = din("recvTo", [C, NO])
    res_rTo_d = din("res_rTo", [C, NO])
    maskT_d = din("maskT", [N, NO], FP8)
    wpack_d = din("wpack", [C, 9 * C + 7])
    idb_d = din("idb", [C, C], BF16)
    idb8_d = din("idb8", [C, C], FP8)
    wprojb_d = din("wprojb", [C, C], BF16)
    onesrow_d = din("ones_row", [1, C])

    scr_wms_d = nc.dram_tensor("scr_wms", [2, N], F32).ap()
    outT_d = dout("outT", [C, NO])
    vr2T_d = dout("vr2T", [C, NO])
    vs2T_d = dout("vs2T", [C, NO])

    def f32v(ap):
        return ap.bitcast(F32)

    with tile.TileContext(nc) as tc, ExitStack() as ctx:
        const = ctx.enter_context(tc.tile_pool(name="const", bufs=1))
        per = ctx.enter_context(tc.tile_pool(name="per", bufs=1))
        stat = ctx.enter_context(tc.tile_pool(name="stat", bufs=1))
        stmp = ctx.enter_context(tc.tile_pool(name="stmp", bufs=3))
        rtmp = ctx.enter_context(tc.tile_pool(name="rtmp", bufs=3))
        ck = ctx.enter_context(tc.tile_pool(name="ck", bufs=2))
        strm = ctx.enter_context(tc.tile_pool(name="strm", bufs=5))
        lcp = ctx.enter_context(tc.tile_pool(name="lc", bufs=3))
        tccp = ctx.enter_context(tc.tile_pool(name="tcc", bufs=4))
        ucp = ctx.enter_context(tc.tile_pool(name="uc", bufs=4))
        pp = ctx.enter_context(tc.tile_pool(name="pp", bufs=4))
        pttp = ctx.enter_context(tc.tile_pool(name="ptt", bufs=4))
        # PSUM is 8 bank-granular buffers total: 4 shared transient banks
        # (matmuls, transposes, row matmuls, projections) + 2 pv packs
        # (4 x [C,128] subtiles each) + 2 A/MS/den packs.
        ps_mm = ctx.enter_context(tc.tile_pool(name="ps_mm", bufs=4, space="PSUM"))
        ps_pv = ctx.enter_context(tc.tile_pool(name="ps_pv", bufs=2, space="PSUM"))
        ps_dn = ctx.enter_context(tc.tile_pool(name="ps_dn", bufs=2, space="PSUM"))
        ps_tp = ps_mm
        ps_rw = ps_mm

        # pin the Ln/Exp table (set 6); Square/Copy/Identity live in every set
        nc.scalar.add_instruction(mybir.InstLoadActFuncSet(
            name=nc.get_next_instruction_name(), engine=mybir.EngineType.Activation,
            act_func_set_id=6, ins=[], outs=[]))

        # ---------------- constants ----------------
        wpack = const.tile([C, 9 * C + 7], F32R, tag="wpack", name="wpack")
        nc.sync.dma_start(wpack[:, 0:4 * C], wpack_d[:, 0:4 * C].bitcast(F32R))
        w_send = wpack[:, 0 * C:1 * C]
        w_mem = wpack[:, 1 * C:2 * C]
        w_recv = wpack[:, 2 * C:3 * C]
        w_qs = wpack[:, 3 * C:4 * C]
        w_proj = wpack[:, 4 * C:5 * C]
        w_r = wpack[:, 5 * C:6 * C]
        w_s = wpack[:, 6 * C:7 * C]
        idf = wpack[:, 7 * C:8 * C]
        jc_w = wpack[:, 8 * C:9 * C]              # J/C (centering broadcast)
        bp = f32v(wpack[:, 9 * C + 0:9 * C + 1])
        br_c = f32v(wpack[:, 9 * C + 1:9 * C + 2])
        bs_c = f32v(wpack[:, 9 * C + 2:9 * C + 3])
        oneD = wpack[:, 9 * C + 3:9 * C + 4]      # 1/C
        one = wpack[:, 9 * C + 4:9 * C + 5]       # 1.0
        lnA_col = f32v(wpack[:, 9 * C + 5:9 * C + 6])
        idb = const.tile([C, C], BF16, tag="idb", name="idb")
        nc.sync.dma_start(idb[:], idb_d)
        idb8 = const.tile([C, C], FP8, tag="idb8", name="idb8")
        nc.sync.dma_start(idb8[:], idb8_d)
        wprojb = const.tile([C, C], BF16, tag="wprojb", name="wprojb")
        nc.sync.dma_start(wprojb[:], wprojb_d)
        ones_row = const.tile([1, C], F32R, tag="ones_row", name="ones_row")
        nc.sync.dma_start(ones_row[:], onesrow_d.bitcast(F32R))
        ones_bcol = const.tile([C, 1], BF16, tag="ones_bcol", name="ones_bcol")
        nc.gpsimd.memset(ones_bcol[:], 1.0)

        # persistent tensors
        v_sT = per.tile([C, N], F32R)
        c_sT = v_sT  # centered in place
        v_rT = per.tile([C, NO], F32R)
        qT = per.tile([C, NO], F32R)
        vr_s = per.tile([C, NO], F32R)
        v_r_nat = per.tile([C, ITI * C], BF16)
        v_s_aug = per.tile([C, JT * C], BF16)
        outT_pre = per.tile([C, NO], BF16)

        w_row = stat.tile([1, N], F32)
        u_eps_row = stat.tile([1, NO], F32R)
        alpha_row = stat.tile([1, NO], F32R)
        colsA = stat.tile([C, ITI], F32)          # w columns, own half
        colsB = stat.tile([C, ITI], F32)
        colsAB = (colsA, colsB)
        mball = stat.tile([C, JT * NO], FP8)      # all mask^T tiles, resident

        def stream3(d_ap, half3):
            t = strm.tile([C, 3 * 512], F32R, tag="instream", name="instream")
            nc.sync.dma_start(t[:], d_ap[:, bass.ts(half3, 1536)].bitcast(F32R))
            return t

        # PE clock primer
        warm = stmp.tile([C, 512], BF16, tag="warm", name="warm")
        nc.gpsimd.memset(warm[:], 0.0)
        for _ in range(6):
            pswarm = ps_mm.tile([C, 512], F32, tag="mm", name="mm")
            nc.tensor.matmul(pswarm[:], warm[:, 0:C], warm[:], start=True, stop=True)

        # -------- phase 1: stage-pipelined across the 3 chunks of a half ----
        # (per-chunk serial chains cost ~5us each in sem hops; emitting each
        # stage for all 3 chunks keeps every engine fed with independent work)
        def sender_half(xc, sc, rc, h3):
            jcs = [h3 * ICH + k for k in range(ICH)]
            ps_t, psb_t, sq_t, vsb_t = {}, {}, {}, {}
            for k, jc in enumerate(jcs):
                lsl = bass.ts(k, 512)
                ps = ps_mm.tile([C, 512], F32, tag="mm", name="mm")
                nc.tensor.matmul(ps[:], w_send, xc[:, lsl], start=True, stop=False)
                nc.tensor.matmul(ps[:], w_mem, sc[:, lsl], start=False, stop=True)
                ps_t[k] = ps
            for k, jc in enumerate(jcs):
                sl, lsl = bass.ts(jc, 512), bass.ts(k, 512)
                nc.vector.tensor_tensor(out=v_sT[:, sl], in0=ps_t[k][:], in1=f32v(rc[:, lsl]), op=ALU.add)
            if h3 == 0:
                # vs2 projection must read UNCENTERED v_s (centered in place below)
                for k, jc in enumerate(jcs):
                    sl = bass.ts(jc, 512)
                    psj = ps_mm.tile([C, 512], F32, tag="mm", name="mm")
                    nc.tensor.matmul(psj[:], w_s, v_sT[:, sl], start=True, stop=True)
                    ob = stmp.tile([C, 512], F32, tag="ob", name="ob")
                    nc.scalar.activation(ob[:], psj[:], AF.Identity, bias=bs_c)
                    nc.sync.dma_start(vs2T_d[:, sl], ob[:])
            for k, jc in enumerate(jcs):
                sl = bass.ts(jc, 512)
                psb = ps_mm.tile([C, 512], F32, tag="mm", name="mm")
                nc.tensor.matmul(psb[:], jc_w, v_sT[:, sl], start=True, stop=True)
                psb_t[k] = psb
            for k, jc in enumerate(jcs):
                sl = bass.ts(jc, 512)
                nc.vector.tensor_tensor(out=c_sT[:, sl], in0=f32v(v_sT[:, sl]), in1=psb_t[k][:], op=ALU.subtract)
            for k, jc in enumerate(jcs):
                sl = bass.ts(jc, 512)
                sqc = ck.tile([C, 512], F32R, tag="sqc", name="sqc", bufs=4)
                nc.scalar.activation(sqc[:], f32v(c_sT[:, sl]), AF.Square)
                sq_t[k] = sqc
                vsb = ck.tile([C, 512], BF16, tag="vsb", name="vsb", bufs=3)
                nc.gpsimd.tensor_copy(vsb[:], f32v(c_sT[:, sl]))
                vsb_t[k] = vsb
            for k, jc in enumerate(jcs):
                sl = bass.ts(jc, 512)
                psw = ps_rw.tile([1, 512], F32, tag="mm", name="row")[:]
                nc.tensor.matmul(psw, oneD, sq_t[k][:], start=True, stop=True)
                nc.scalar.copy(w_row[0:1, sl], psw)
                pst = ps_tp.tile([C, 512], BF16, tag="mm", name="tp")
                for t in range(4):
                    nc.tensor.transpose(pst[:, bass.ts(t, 128)], vsb_t[k][:, bass.ts(t, 128)], idb[:])
                nc.vector.tensor_copy(v_s_aug[:, sl], pst[:])
            for k, jc in enumerate(jcs):
                sl = bass.ts(jc, 512)
                t0 = jc % ICH
                nc.sync.dma_start(scr_wms_d[0:1, sl], w_row[0:1, sl])
                nc.sync.dma_start(
                    colsAB[h3][:, t0 * 4:t0 * 4 + 4],
                    scr_wms_d[0:1, sl].rearrange("o (t p) -> (o p) t", p=128))

        def receiver_half(xc, rcv, rrc):
            ps_t, psb_t, cr_t, q_t, sq_t, qv_t = {}, {}, {}, {}, {}, {}
            R = range(ICH)
            for k in R:
                sl = bass.ts(k, 512)
                ps2 = ps_mm.tile([C, 512], F32, tag="mm", name="mm")
                nc.tensor.matmul(ps2[:], w_recv, xc[:, sl], start=True, stop=False)
                nc.tensor.matmul(ps2[:], w_mem, rcv[:, sl], start=False, stop=True)
                ps_t[k] = ps2
            for k in R:
                sl = bass.ts(k, 512)
                nc.vector.tensor_tensor(out=v_rT[:, sl], in0=ps_t[k][:], in1=f32v(rrc[:, sl]), op=ALU.add)
            for k in R:
                sl = bass.ts(k, 512)
                psb = ps_mm.tile([C, 512], F32, tag="mm", name="mm")
                nc.tensor.matmul(psb[:], jc_w, v_rT[:, sl], start=True, stop=True)
                psb_t[k] = psb
            for k in R:
                sl = bass.ts(k, 512)
                cr = ck.tile([C, 512], F32R, tag="cr", name="cr", bufs=3)
                nc.vector.tensor_tensor(out=cr[:], in0=f32v(v_rT[:, sl]), in1=psb_t[k][:], op=ALU.subtract)
                cr_t[k] = cr
            for k in R:
                sl = bass.ts(k, 512)
                ps3 = ps_mm.tile([C, 512], F32, tag="mm", name="mm")
                nc.tensor.matmul(ps3[:], w_qs, xc[:, sl], start=True, stop=True)
                q_t[k] = ps3
                nc.vector.tensor_scalar_mul(vr_s[:, sl], f32v(cr_t[k][:]), 2.0 / C)
            for k in R:
                sl = bass.ts(k, 512)
                nc.scalar.copy(qT[:, sl], q_t[k][:])
                sqr = ck.tile([C, 512], F32R, tag="sqc", name="sqc", bufs=4)
                nc.scalar.activation(sqr[:], f32v(cr_t[k][:]), AF.Square)
                psq = ps_rw.tile([1, 512], F32, tag="mm", name="row")[:]
                nc.tensor.matmul(psq, oneD, sqr[:], start=True, stop=True)
                nc.vector.tensor_scalar(out=u_eps_row[0:1, sl], in0=psq,
                                        scalar1=1.0, scalar2=EPS, op0=ALU.mult, op1=ALU.add)
            for k in R:
                sl = bass.ts(k, 512)
                qv = ck.tile([C, 512], F32R, tag="sqc", name="sqc", bufs=4)
                nc.vector.tensor_tensor(out=qv[:], in0=f32v(qT[:, sl]), in1=f32v(cr_t[k][:]), op=ALU.mult)
                psa = ps_rw.tile([1, 512], F32, tag="mm", name="row")[:]
                nc.tensor.matmul(psa, one, qv[:], start=True, stop=True)
                nc.scalar.copy(alpha_row[0:1, sl], psa)
            for k in R:
                sl = bass.ts(k, 512)
                pst = ps_tp.tile([C, 512], F32R, tag="mm", name="tp")
                for t in range(4):
                    nc.tensor.transpose(pst[:, bass.ts(t, 128)], cr_t[k][:, bass.ts(t, 128)], idf)
                nc.vector.tensor_copy(v_r_nat[:, sl], f32v(pst[:]))
            for k in R:
                sl = bass.ts(k, 512)
                psj = ps_mm.tile([C, 512], F32, tag="mm", name="mm")
                nc.tensor.matmul(psj[:], w_r, v_rT[:, sl], start=True, stop=True)
                ob = stmp.tile([C, 512], F32, tag="ob", name="ob")
                nc.scalar.activation(ob[:], psj[:], AF.Identity, bias=br_c)
                nc.sync.dma_start(vr2T_d[:, sl], ob[:])

        def w_col(jt):
            h, t = divmod(jt, ITI)
            return colsAB[h][:, t:t + 1]

        # -------- attention --------
        def load_mask(jt):
            nc.sync.dma_start(mball[:, jt * NO:(jt + 1) * NO], maskT_d[bass.ts(jt, 128), :])

        def emit_attn(ic, jt, pv, amsden, first, last):
            isl = bass.ts(ic, 512)
            jsl = bass.ts(jt, 128)
            mk = mball[:, jt * NO + ic * 512: jt * NO + ic * 512 + 512]

            ps_v = ps_mm.tile([C, 512], F32, tag="mm", name="mm")
            nc.tensor.matmul(ps_v[:], c_sT[:, jsl], vr_s[:, isl], start=True, stop=False)
            nc.tensor.matmul(ps_v[:], ones_row[0:1, 0:C], u_eps_row[0:1, isl], start=False, stop=True)
            ps_s = ps_mm.tile([C, 512], F32, tag="mm", name="mm")
            nc.tensor.matmul(ps_s[:], c_sT[:, jsl], qT[:, isl], start=True, stop=False)
            nc.tensor.matmul(ps_s[:], idb8[:], mk, start=False, stop=False)
            nc.tensor.matmul(ps_s[:], ones_row[0:1, 0:C], alpha_row[0:1, isl], start=False, stop=True)

            lc = lcp.tile([C, 512], F32, tag="lc", name="lc")
            nc.scalar.activation(lc[:], ps_v[:], AF.Ln, bias=w_col(jt))
            tcc = tccp.tile([C, 512], FP16, tag="tcc", name="tcc")
            nc.scalar.activation(tcc[:], lc[:], AF.Exp, scale=-0.5, bias=lnA_col)
            uc = ucp.tile([C, 512], FP16, tag="uc", name="uc")
            nc.vector.tensor_tensor(out=uc[:], in0=ps_s[:], in1=tcc[:], op=ALU.mult)
            pt = pp.tile([C, 512], I16, tag="pt", name="pt")
            nc.vector.tensor_scalar(out=pt[:], in0=uc[:], scalar1=UCCLAMP,
                                    scalar2=BEXP - DELTA, op0=ALU.max, op1=ALU.add)
            pb = pt[:].bitcast(BF16)
            ptt = pttp.tile([C, 512], BF16, tag="ptt", name="ptt")
            nc.vector.tensor_tensor(out=ptt[:], in0=pb, in1=tcc[:], op=ALU.mult)
            aug = v_s_aug[:, jsl]
            # start=True zeroes the ENTIRE psum bank, so each bank (pv; amsden)
            # is started exactly once: by its first chain's first matmul. All
            # other chains begin on pending-zeroed bytes.
            for t in range(4):
                tsl = bass.ts(t, 128)
                nc.tensor.matmul(pv[:, tsl], ptt[:, tsl], aug,
                                 start=(first and t == 0), stop=last,
                                 skip_group_check=True)
                nc.tensor.matmul(amsden[:, 2 * t + 0:2 * t + 1], ptt[:, tsl], ones_bcol[:],
                                 start=(first and t == 0), stop=last,
                                 skip_group_check=True)
                nc.tensor.matmul(amsden[:, 2 * t + 1:2 * t + 2], pb[:, tsl], ones_bcol[:],
                                 start=False, stop=last, skip_group_check=True)

        def emit_epilogue_tile(ic, t, pv, amsden):
            it = ic * 4 + t
            acol = amsden[:, 2 * t + 0:2 * t + 1]
            dcol = amsden[:, 2 * t + 1:2 * t + 2]
            rcol = stmp.tile([C, 1], F32, tag="rcol", name="rcol")
            nc.vector.reciprocal_approx_fast(out=rcol[:], in_=dcol)
            x1 = stmp.tile([C, C], F32, tag="x1", name="x1")
            nc.vector.scalar_tensor_tensor(
                out=x1[:], in0=v_r_nat[:, bass.ts(it, 128)], scalar=acol,
                in1=pv[:, bass.ts(t, 128)], op0=ALU.mult, op1=ALU.add)
            x2 = stmp.tile([C, C], BF16, tag="x2", name="x2")
            nc.vector.tensor_scalar(
                out=x2[:], in0=x1[:], scalar1=rcol[:, 0:1], scalar2=1.0 / AEXP,
                op0=ALU.mult, op1=ALU.mult)
            pso = ps_tp.tile([C, C], BF16, tag="mm", name="tp")
            nc.tensor.transpose(pso[:], x2[:], idb[:])
            nc.vector.tensor_copy(outT_pre[:, bass.ts(it, 128)], pso[:])

        def emit_epilogue_proj(ic):
            isl = bass.ts(ic, 512)
            pspj = ps_mm.tile([C, 512], F32, tag="mm", name="mm")
            nc.tensor.matmul(pspj[:], wprojb[:], outT_pre[:, isl], start=True, stop=True)
            obj = stmp.tile([C, 512], F32, tag="ob", name="ob")
            nc.scalar.activation(obj[:], pspj[:], AF.Identity, bias=bp)
            nc.sync.dma_start(outT_d[:, isl], obj[:])

        # ---- emission ----
        xc0 = stream3(xT_d, 0)
        sc0 = stream3(sendT_d, 0)
        rc0 = stream3(res_sT_d, 0)
        rcv = strm.tile([C, 1536], F32R, tag="instream", name="rcv")
        nc.sync.dma_start(rcv[:], recvTo_d.bitcast(F32R))
        # (idb/idb8/ones already queued above; wpack-b and masks follow)
        rrc = strm.tile([C, 1536], F32R, tag="instream", name="rrc")
        nc.sync.dma_start(rrc[:], res_rTo_d.bitcast(F32R))
        nc.sync.dma_start(wpack[:, 4 * C:], wpack_d[:, 4 * C:].bitcast(F32R))
        sender_half(xc0, sc0, rc0, 0)
        receiver_half(xc0, rcv, rrc)
        for jt in range(ITI):
            load_mask(jt)
        xc1 = stream3(xT_d, 1)
        sc1 = stream3(sendT_d, 1)
        rc1 = stream3(res_sT_d, 1)
        for jt in range(ITI, JT):
            load_mask(jt)
        sender_half(xc1, sc1, rc1, 1)

        # attention: sequential ics; epilogue pieces spread one-per-iteration
        # into the next ic so they never clog an engine queue.
        pvs, dens = {}, {}
        epi_q = []
        for ic in range(ICH):
            pvs[ic] = ps_pv.tile([C, 512], F32, tag="pv", name="pv")
            dens[ic] = ps_dn.tile([C, 8], F32, tag="den", name="den")
            for jt in range(JT):
                emit_attn(ic, jt, pvs[ic], dens[ic], first=(jt == 0), last=(jt == JT - 1))
                if jt >= 2 and epi_q:
                    eic, et = epi_q.pop(0)
                    if et == "proj":
                        emit_epilogue_proj(eic)
                    else:
                        emit_epilogue_tile(eic, et, pvs[eic], dens[eic])
            epi_q.extend((ic, t) for t in range(4))
            epi_q.append((ic, "proj"))
        for eic, et in epi_q:
            if et == "proj":
                emit_epilogue_proj(eic)
            else:
                emit_epilogue_tile(eic, et, pvs[eic], dens[eic])

    nc.compile()
    return nc


def _host_prep(inputs):
    """Returns list of 8 per-core input dicts."""
    f32 = np.float32
    fp8np = mybir.dt.np(FP8)
    x = np.asarray(inputs["x"], f32)
    recv = np.asarray(inputs["receiver_val_res"], f32)
    res_r = np.asarray(inputs["residual_receiver"], f32)
    send = np.asarray(inputs["sender_val_res"], f32)
    res_s = np.asarray(inputs["residual_sender"], f32)
    mask = np.asarray(inputs["attn_mask"])
    ra = np.asarray(inputs["relation_attn"], f32)
    q_w = np.asarray(inputs["q_w"], f32)
    proj_w = np.asarray(inputs["proj_w"], f32)
    proj_b = np.asarray(inputs["proj_b"], f32)
    r_w = np.asarray(inputs["r_w"], f32)
    r_b = np.asarray(inputs["r_b"], f32)
    s_w = np.asarray(inputs["s_w"], f32)
    s_b = np.asarray(inputs["s_b"], f32)
    n_weight = np.asarray(inputs["n_weight"], f32)
    n_bias = np.asarray(inputs["n_bias"], f32)

    mem_w, recv_w, send_w = ra[:, :C], ra[:, C:2 * C], ra[:, 2 * C:]
    w_proj_eff = proj_w * n_weight[None, :]
    b_proj_eff = proj_w @ n_bias + proj_b

    cc = np.ascontiguousarray
    wpack = np.concatenate([
        send_w.T, mem_w.T, recv_w.T, q_w.T * SCALE, w_proj_eff.T, r_w.T, s_w.T,
        np.eye(C, dtype=f32), np.full((C, C), 1.0 / C, f32),
        b_proj_eff[:, None], r_b[:, None], s_b[:, None],
        np.full((C, 1), 1.0 / C, f32), np.ones((C, 1), f32),
        np.full((C, 1), LNA, f32), np.zeros((C, 1), f32),
    ], axis=1).astype(f32)
    weights = {
        "wpack": cc(wpack),
        "idb": cc(np.eye(C).astype(ml_dtypes.bfloat16)),
        "idb8": cc(np.eye(C).astype(fp8np)),
        "wprojb": cc(w_proj_eff.T.astype(ml_dtypes.bfloat16)),
        "ones_row": np.ones((1, C), f32),
    }

    in_maps = []
    for core in range(8):
        b, half = core // 2, core % 2
        i0, i1 = half * NO, (half + 1) * NO
        # sender-axis permutation: own half first (program assumes own = [0, NO))
        perm = np.concatenate([np.arange(i0, i1), np.arange(0, i0), np.arange(i1, N)])
        xb = x[:, b, :].T                          # [C, N]
        mT = mask[b, 0, i0:i1, :].T.astype(f32)    # [N(j), NO(i)]
        m = {
            "xT": cc(xb[:, perm]),
            "sendT": cc(send[:, b, :].T[:, perm]),
            "res_sT": cc(res_s[:, b, :].T[:, perm]),
            "recvTo": cc(recv[i0:i1, b, :].T),
            "res_rTo": cc(res_r[i0:i1, b, :].T),
            "maskT": cc((mT[perm, :] * (-MASKM)).astype(fp8np)),
        }
        m.update(weights)
        in_maps.append(m)
    return in_maps


def kernel(**inputs):
    if "nc" not in _CACHE:
        _CACHE["nc"] = _build_program()
    nc = _CACHE["nc"]
    in_maps = _host_prep(inputs)
    res = run_bass_kernel_spmd(nc, in_maps, core_ids=list(range(8)))
    out = np.zeros((N, B, C), np.float32)
    vr2 = np.zeros((N, B, C), np.float32)
    vs2 = np.zeros((N, B, C), np.float32)
    for core in range(8):
        b, half = core // 2, core % 2
        i0, i1 = half * NO, (half + 1) * NO
        r = res.results[core]
        out[i0:i1, b, :] = r["outT"].T
        vr2[i0:i1, b, :] = r["vr2T"].T
        vs2[i0:i1, b, :] = r["vs2T"].T
    return out, vr2, vs2


# revision 3
# speedup vs baseline: 1.0087x; 1.0087x over previous
"""Trainium2 Bass kernel v2 for nn_AttentionTIE (TIE-style edge-LayerNorm attention).

Sharding: 8 cores = (batch b = core//2) x (receiver-row half = core%2).

v2 redesign vs v1 (cost-model: ACT 139us / DVE 136us / PE 116us busy, 188us wall):
  - softmax exp moved OFF the ACT engine onto the DVE via the Schraudolph
    bit trick: P = bitcast_bf16(int16(A*uc + B - delta)), A = 128/ln2.
    One 4x-mode tensor_scalar (max-clamp + bias add) replaces the ACT Exp
    pass + accumulator read. ACT keeps only Ln+Exp for T = A/sqrt(std2)
    (Rsqrt tables are banned in bass; Ln/Exp share table set 6).
  - whole score/std2 pipeline computed TRANSPOSED (j on partitions,
    "B layout"): w_var rides the Ln bias (per-partition), u_eps and alpha
    enter as rank-1 matmuls, the mask (host-prepped fp8 = -60*mask^T) via
    an identity matmul. P^T comes out of the exp directly - no PE
    transposes of P, and PT^T = P^T * tccT is ONE 2x tensor_tensor.
  - denominator = sum_j P via 1-wide PE matmuls (P^T lhsT @ ones): output
    width 1 => ~1 cycle each on the cost model, exact f32 PSUM accumulation.
  - uc kept in fp16 (not bf16): halves the exp-argument quantization error
    at identical DVE cost (4x mode needs any 2-byte dtype).
  - epilogue reciprocal via reciprocal_approx_fast (v1 lost 8.5us to
    bit-exact InstReciprocal on [128,1] columns).
  - phase 1: own-half v_s reuse (v1 recomputed it from re-streamed inputs),
    single-matmul m_s, row-ops spread over Pool/ACT.

Layout summary (all tiles [partition, free]):
  v_sT=[C,N] f32r (centered in place); v_s_aug=[j,(C+2)*24] bf16 (v_s|1|m_s)
  per (ic: 3 x 512 receiver cols, jt: 24 x 128 sender rows):
    ps_v[j,i] = c_sT[jt]^T vr_s + 1 (x) u_eps        (u via rank-1)
    ps_s[j,i] = c_sT[jt]^T qT + maskT + 1 (x) alpha  (mask fp8 idb-matmul)
    tccT = Exp(-.5*Ln(ps_v + w_col) + lnA)  -> fp16  (= A/std)
    uc   = ps_s * tccT                      -> fp16  (DVE tt)
    P^T  = int16(max(uc,-13000) + B-delta)  -> bf16 bits (DVE ts, 4x)
    PTT  = P^T * tccT                       -> bf16  (DVE tt, 2x)
    pv[i,0:C+2] += PTT^T @ v_s_aug[jt]; den[i] += P^T @ 1
  epilogue: out = (pv + A_i v_r - (m_r A_i + MS)) / (A*den); three
  projections as in v1.
"""
import sys

sys.path.insert(0, "/opt/trn_rl_repo")

import numpy as np
import ml_dtypes  # noqa: E402
from contextlib import ExitStack  # noqa: E402

import concourse.bass as bass  # noqa: E402
import concourse.tile as tile  # noqa: E402
from concourse import bacc  # noqa: E402
from concourse import mybir  # noqa: E402
from concourse.bass_utils import run_bass_kernel_spmd  # noqa: E402

N, B, C = 3072, 4, 128
NO = N // 2          # own receiver rows per core
ITI = NO // 128      # 12 i-tiles
ICH = NO // 512      # 3 i-chunks (512)
JT = N // 128        # 24 j-tiles
JCH = N // 512       # 6 sender chunks
EPS = 1e-5
SCALE = C ** -0.5
MASKM = 60.0
AEXP = 128.0 / float(np.log(2.0))     # 184.6649652
LNA = float(np.log(AEXP))             # 5.21868...
BEXP = 127.0 * 128.0                  # 16256
DELTA = 5.1                           # floor-calibrated Schraudolph bias
UCCLAMP = -13000.0

F32 = mybir.dt.float32
F32R = mybir.dt.float32r
BF16 = mybir.dt.bfloat16
FP16 = mybir.dt.float16
FP8 = mybir.dt.float8e4
I16 = mybir.dt.int16
AF = mybir.ActivationFunctionType
ALU = mybir.AluOpType
AX = mybir.AxisListType

_CACHE = {}


def _build_program():
    """One program for all cores: the host permutes the sender axis so each
    core's own receiver half is always sender columns [0, NO)."""
    nc = bacc.Bacc("TRN2", target_bir_lowering=False, debug=False, num_devices=8)

    def din(name, shape, dtype=F32):
        return nc.dram_tensor(name, list(shape), dtype, kind="ExternalInput").ap()

    def dout(name, shape, dtype=F32):
        return nc.dram_tensor(name, list(shape), dtype, kind="ExternalOutput").ap()

    xT_d = din("xT", [C, N], BF16)
    sendT_d = din("sendT", [C, N], BF16)
    res_sT_d = din("res_sT", [C, N], BF16)
    recvTo_d = din("recvTo", [C, NO], BF16)
    res_rTo_d = din("res_rTo", [C, NO], BF16)
    maskT_d = din("maskT", [N, NO], FP8)
    wpack_d = din("wpack", [C, 9 * C + 7])
    idb_d = din("idb", [C, C], BF16)
    idb8_d = din("idb8", [C, C], FP8)
    wprojb_d = din("wprojb", [C, C], BF16)
    wbf_d = din("wbf", [C, 4 * C], BF16)
    onesrow_d = din("ones_row", [1, C])

    scr_wms_d = nc.dram_tensor("scr_wms", [2, N], F32).ap()
    outT_d = dout("outT", [C, NO], BF16)
    vr2T_d = dout("vr2T", [C, NO], BF16)
    vs2T_d = dout("vs2T", [C, NO], BF16)

    def f32v(ap):
        return ap.bitcast(F32)

    with tile.TileContext(nc) as tc, ExitStack() as ctx:
        const = ctx.enter_context(tc.tile_pool(name="const", bufs=1))
        per = ctx.enter_context(tc.tile_pool(name="per", bufs=1))
        stat = ctx.enter_context(tc.tile_pool(name="stat", bufs=1))
        stmp = ctx.enter_context(tc.tile_pool(name="stmp", bufs=3))
        rtmp = ctx.enter_context(tc.tile_pool(name="rtmp", bufs=3))
        ck = ctx.enter_context(tc.tile_pool(name="ck", bufs=2))
        strm = ctx.enter_context(tc.tile_pool(name="strm", bufs=5))
        lcp = ctx.enter_context(tc.tile_pool(name="lc", bufs=3))
        tccp = ctx.enter_context(tc.tile_pool(name="tcc", bufs=4))
        ucp = ctx.enter_context(tc.tile_pool(name="uc", bufs=4))
        pp = ctx.enter_context(tc.tile_pool(name="pp", bufs=4))
        pttp = ctx.enter_context(tc.tile_pool(name="ptt", bufs=4))
        pt8p = ctx.enter_context(tc.tile_pool(name="pt8", bufs=2))
        # PSUM is 8 bank-granular buffers total: 4 shared transient banks
        # (matmuls, transposes, row matmuls, projections) + 2 pv packs
        # (4 x [C,128] subtiles each) + 2 A/MS/den packs.
        ps_mm = ctx.enter_context(tc.tile_pool(name="ps_mm", bufs=4, space="PSUM"))
        ps_pv = ctx.enter_context(tc.tile_pool(name="ps_pv", bufs=2, space="PSUM"))
        ps_dn = ctx.enter_context(tc.tile_pool(name="ps_dn", bufs=2, space="PSUM"))
        ps_tp = ps_mm
        ps_rw = ps_mm

        # pin the Ln/Exp table (set 6); Square/Copy/Identity live in every set
        nc.scalar.add_instruction(mybir.InstLoadActFuncSet(
            name=nc.get_next_instruction_name(), engine=mybir.EngineType.Activation,
            act_func_set_id=6, ins=[], outs=[]))

        # ---------------- constants ----------------
        wpack = const.tile([C, 9 * C + 7], F32R, tag="wpack", name="wpack")
        nc.sync.dma_start(wpack[:, 0:4 * C], wpack_d[:, 0:4 * C].bitcast(F32R))
        w_send = wpack[:, 0 * C:1 * C]
        w_mem = wpack[:, 1 * C:2 * C]
        w_recv = wpack[:, 2 * C:3 * C]
        w_qs = wpack[:, 3 * C:4 * C]
        w_proj = wpack[:, 4 * C:5 * C]
        w_r = wpack[:, 5 * C:6 * C]
        w_s = wpack[:, 6 * C:7 * C]
        idf = wpack[:, 7 * C:8 * C]
        jc_w = wpack[:, 8 * C:9 * C]              # J/C (centering broadcast)
        bp = f32v(wpack[:, 9 * C + 0:9 * C + 1])
        br_c = f32v(wpack[:, 9 * C + 1:9 * C + 2])
        bs_c = f32v(wpack[:, 9 * C + 2:9 * C + 3])
        oneD = wpack[:, 9 * C + 3:9 * C + 4]      # 1/C
        one = wpack[:, 9 * C + 4:9 * C + 5]       # 1.0
        lnA_col = f32v(wpack[:, 9 * C + 5:9 * C + 6])
        idb = const.tile([C, C], BF16, tag="idb", name="idb")
        nc.sync.dma_start(idb[:], idb_d)
        idb8 = const.tile([C, C], FP8, tag="idb8", name="idb8")
        nc.sync.dma_start(idb8[:], idb8_d)
        wprojb = const.tile([C, C], BF16, tag="wprojb", name="wprojb")
        nc.sync.dma_start(wprojb[:], wprojb_d)
        wbf = const.tile([C, 4 * C], BF16, tag="wbf", name="wbf")
        nc.sync.dma_start(wbf[:], wbf_d)
        wb_send = wbf[:, 0 * C:1 * C]
        wb_mem = wbf[:, 1 * C:2 * C]
        wb_recv = wbf[:, 2 * C:3 * C]
        wb_qs = wbf[:, 3 * C:4 * C]
        ones_row = const.tile([1, C], F32R, tag="ones_row", name="ones_row")
        nc.sync.dma_start(ones_row[:], onesrow_d.bitcast(F32R))
        ones_bcol = const.tile([C, 1], BF16, tag="ones_bcol", name="ones_bcol")
        nc.gpsimd.memset(ones_bcol[:], 1.0)
        ones8 = const.tile([C, 2], FP8, tag="ones8", name="ones8")
        nc.gpsimd.memset(ones8[:], 1.0)

        # persistent tensors
        v_sT = per.tile([C, N], F32R)
        c_sT = v_sT  # centered in place
        v_rT = per.tile([C, NO], F32R)
        qT = per.tile([C, NO], F32R)
        vr_s = per.tile([C, NO], F32R)
        v_r_nat = per.tile([C, ITI * C], BF16)
        v_s_aug = per.tile([C, JT * C], FP8)
        outT_pre = per.tile([C, NO], BF16)

        w_row = stat.tile([1, N], F32)
        u_eps_row = stat.tile([1, NO], F32R)
        alpha_row = stat.tile([1, NO], F32R)
        colsA = stat.tile([C, ITI], F32)          # w columns, own half
        colsB = stat.tile([C, ITI], F32)
        colsAB = (colsA, colsB)
        mball = stat.tile([C, JT * NO], FP8)      # all mask^T tiles, resident

        def stream3(d_ap, half3):
            t = strm.tile([C, 3 * 512], BF16, tag="instream", name="instream")
            nc.sync.dma_start(t[:], d_ap[:, bass.ts(half3, 1536)])
            return t

        # PE clock primer
        warm = stmp.tile([C, 512], BF16, tag="warm", name="warm")
        nc.gpsimd.memset(warm[:], 0.0)
        for _ in range(6):
            pswarm = ps_mm.tile([C, 512], F32, tag="mm", name="mm")
            nc.tensor.matmul(pswarm[:], warm[:, 0:C], warm[:], start=True, stop=True)

        # -------- phase 1: stage-pipelined across the 3 chunks of a half ----
        # (per-chunk serial chains cost ~5us each in sem hops; emitting each
        # stage for all 3 chunks keeps every engine fed with independent work)
        def sender_half_gen(xc, sc, rc, h3):
            jcs = [h3 * ICH + k for k in range(ICH)]
            ps_t, psb_t, sq_t, vsb_t = {}, {}, {}, {}
            for k, jc in enumerate(jcs):
                lsl = bass.ts(k, 512)
                ps = ps_mm.tile([C, 512], F32, tag="mm", name="mm")
                nc.tensor.matmul(ps[:], wb_send, xc[:, lsl], start=True, stop=False)
                nc.tensor.matmul(ps[:], wb_mem, sc[:, lsl], start=False, stop=True)
                ps_t[k] = ps
            yield
            for k, jc in enumerate(jcs):
                sl, lsl = bass.ts(jc, 512), bass.ts(k, 512)
                nc.vector.tensor_tensor(out=v_sT[:, sl], in0=ps_t[k][:], in1=rc[:, lsl], op=ALU.add)
            yield
            if h3 == 0:
                # vs2 projection must read UNCENTERED v_s (centered in place below)
                for k, jc in enumerate(jcs):
                    sl = bass.ts(jc, 512)
                    psj = ps_mm.tile([C, 512], F32, tag="mm", name="mm")
                    nc.tensor.matmul(psj[:], w_s, v_sT[:, sl], start=True, stop=True)
                    ob = stmp.tile([C, 512], BF16, tag="ob", name="ob")
                    nc.scalar.activation(ob[:], psj[:], AF.Identity, bias=bs_c)
                    nc.sync.dma_start(vs2T_d[:, sl], ob[:])
            for k, jc in enumerate(jcs):
                sl = bass.ts(jc, 512)
                psb = ps_mm.tile([C, 512], F32, tag="mm", name="mm")
                nc.tensor.matmul(psb[:], jc_w, v_sT[:, sl], start=True, stop=True)
                psb_t[k] = psb
            yield
            for k, jc in enumerate(jcs):
                sl = bass.ts(jc, 512)
                nc.vector.tensor_tensor(out=c_sT[:, sl], in0=f32v(v_sT[:, sl]), in1=psb_t[k][:], op=ALU.subtract)
            yield
            for k, jc in enumerate(jcs):
                sl = bass.ts(jc, 512)
                sqc = ck.tile([C, 512], F32R, tag="sqc", name="sqc", bufs=4)
                nc.scalar.activation(sqc[:], f32v(c_sT[:, sl]), AF.Square)
                sq_t[k] = sqc
                vsb = ck.tile([C, 512], BF16, tag="vsb", name="vsb", bufs=3)
                nc.gpsimd.tensor_copy(vsb[:], f32v(c_sT[:, sl]))
                vsb_t[k] = vsb
            yield
            for k, jc in enumerate(jcs):
                sl = bass.ts(jc, 512)
                psw = ps_rw.tile([1, 512], F32, tag="mm", name="row")[:]
                nc.tensor.matmul(psw, oneD, sq_t[k][:], start=True, stop=True)
                nc.scalar.copy(w_row[0:1, sl], psw)
                pst = ps_tp.tile([C, 512], BF16, tag="mm", name="tp")
                for t in range(4):
                    nc.tensor.transpose(pst[:, bass.ts(t, 128)], vsb_t[k][:, bass.ts(t, 128)], idb[:])
                nc.vector.tensor_copy(v_s_aug[:, sl], pst[:])
            yield
            for k, jc in enumerate(jcs):
                sl = bass.ts(jc, 512)
                t0 = jc % ICH
                nc.sync.dma_start(scr_wms_d[0:1, sl], w_row[0:1, sl])
                nc.sync.dma_start(
                    colsAB[h3][:, t0 * 4:t0 * 4 + 4],
                    scr_wms_d[0:1, sl].rearrange("o (t p) -> (o p) t", p=128))

        def receiver_half_gen(xc, rcv, rrc):
            ps_t, psb_t, cr_t, q_t, sq_t, qv_t = {}, {}, {}, {}, {}, {}
            R = range(ICH)
            for k in R:
                sl = bass.ts(k, 512)
                ps2 = ps_mm.tile([C, 512], F32, tag="mm", name="mm")
                nc.tensor.matmul(ps2[:], wb_recv, xc[:, sl], start=True, stop=False)
                nc.tensor.matmul(ps2[:], wb_mem, rcv[:, sl], start=False, stop=True)
                ps_t[k] = ps2
            yield
            for k in R:
                sl = bass.ts(k, 512)
                nc.vector.tensor_tensor(out=v_rT[:, sl], in0=ps_t[k][:], in1=rrc[:, sl], op=ALU.add)
            for k in R:
                sl = bass.ts(k, 512)
                psb = ps_mm.tile([C, 512], F32, tag="mm", name="mm")
                nc.tensor.matmul(psb[:], jc_w, v_rT[:, sl], start=True, stop=True)
                psb_t[k] = psb
            yield
            for k in R:
                sl = bass.ts(k, 512)
                cr = ck.tile([C, 512], F32R, tag="cr", name="cr", bufs=3)
                nc.vector.tensor_tensor(out=cr[:], in0=f32v(v_rT[:, sl]), in1=psb_t[k][:], op=ALU.subtract)
                cr_t[k] = cr
            yield
            for k in R:
                sl = bass.ts(k, 512)
                ps3 = ps_mm.tile([C, 512], F32, tag="mm", name="mm")
                nc.tensor.matmul(ps3[:], wb_qs, xc[:, sl], start=True, stop=True)
                q_t[k] = ps3
                nc.vector.tensor_scalar_mul(vr_s[:, sl], f32v(cr_t[k][:]), 2.0 / C)
            yield
            for k in R:
                sl = bass.ts(k, 512)
                nc.scalar.copy(qT[:, sl], q_t[k][:])
                sqr = ck.tile([C, 512], F32R, tag="sqc", name="sqc", bufs=4)
                nc.scalar.activation(sqr[:], f32v(cr_t[k][:]), AF.Square)
                psq = ps_rw.tile([1, 512], F32, tag="mm", name="row")[:]
                nc.tensor.matmul(psq, oneD, sqr[:], start=True, stop=True)
                nc.vector.tensor_scalar(out=u_eps_row[0:1, sl], in0=psq,
                                        scalar1=1.0, scalar2=EPS, op0=ALU.mult, op1=ALU.add)
            yield
            for k in R:
                sl = bass.ts(k, 512)
                qv = ck.tile([C, 512], F32R, tag="sqc", name="sqc", bufs=4)
                nc.vector.tensor_tensor(out=qv[:], in0=f32v(qT[:, sl]), in1=f32v(cr_t[k][:]), op=ALU.mult)
                psa = ps_rw.tile([1, 512], F32, tag="mm", name="row")[:]
                nc.tensor.matmul(psa, one, qv[:], start=True, stop=True)
                nc.scalar.copy(alpha_row[0:1, sl], psa)
            yield
            for k in R:
                sl = bass.ts(k, 512)
                pst = ps_tp.tile([C, 512], F32R, tag="mm", name="tp")
                for t in range(4):
                    nc.tensor.transpose(pst[:, bass.ts(t, 128)], cr_t[k][:, bass.ts(t, 128)], idf)
                nc.vector.tensor_copy(v_r_nat[:, sl], f32v(pst[:]))
            yield
            for k in R:
                sl = bass.ts(k, 512)
                psj = ps_mm.tile([C, 512], F32, tag="mm", name="mm")
                nc.tensor.matmul(psj[:], w_r, v_rT[:, sl], start=True, stop=True)
                ob = stmp.tile([C, 512], BF16, tag="ob", name="ob")
                nc.scalar.activation(ob[:], psj[:], AF.Identity, bias=br_c)
                nc.sync.dma_start(vr2T_d[:, sl], ob[:])

        def w_col(jt):
            h, t = divmod(jt, ITI)
            return colsAB[h][:, t:t + 1]

        # -------- attention --------
        def load_mask(jt):
            nc.sync.dma_start(mball[:, jt * NO:(jt + 1) * NO], maskT_d[bass.ts(jt, 128), :])

        p8_live = [None]

        def emit_attn(ic, jt, pv, amsden, first, last):
            isl = bass.ts(ic, 512)
            jsl = bass.ts(jt, 128)
            mk = mball[:, jt * NO + ic * 512: jt * NO + ic * 512 + 512]

            ps_v = ps_mm.tile([C, 512], F32, tag="mm", name="mm")
            nc.tensor.matmul(ps_v[:], c_sT[:, jsl], vr_s[:, isl], start=True, stop=False)
            nc.tensor.matmul(ps_v[:], ones_row[0:1, 0:C], u_eps_row[0:1, isl], start=False, stop=True)
            ps_s = ps_mm.tile([C, 512], F32, tag="mm", name="mm")
            nc.tensor.matmul(ps_s[:], c_sT[:, jsl], qT[:, isl], start=True, stop=False)
            nc.tensor.matmul(ps_s[:], idb8[:], mk, start=False, stop=False)
            nc.tensor.matmul(ps_s[:], ones_row[0:1, 0:C], alpha_row[0:1, isl], start=False, stop=True)

            lc = lcp.tile([C, 512], F32, tag="lc", name="lc")
            nc.scalar.activation(lc[:], ps_v[:], AF.Ln, bias=w_col(jt))
            tcc = tccp.tile([C, 512], FP16, tag="tcc", name="tcc")
            nc.scalar.activation(tcc[:], lc[:], AF.Exp, scale=-0.5, bias=lnA_col)
            uc = ucp.tile([C, 512], FP16, tag="uc", name="uc")
            nc.vector.tensor_tensor(out=uc[:], in0=ps_s[:], in1=tcc[:], op=ALU.mult)
            pt = pp.tile([C, 512], I16, tag="pt", name="pt")
            # bias folds 1/A into P (P' = exp(uc)/A): keeps PT = P'*tcc and the
            # fp8 PTT pair buffer in range (raw A-scaled PT overflows e4m3)
            nc.vector.tensor_scalar(out=pt[:], in0=uc[:], scalar1=UCCLAMP,
                                    scalar2=BEXP - DELTA - AEXP * LNA,
                                    op0=ALU.max, op1=ALU.add)
            pb = pt[:].bitcast(BF16)
            ptt = pttp.tile([C, 512], BF16, tag="ptt", name="ptt")
            nc.vector.tensor_tensor(out=ptt[:], in0=pb, in1=tcc[:], op=ALU.mult)
            # fp8 PTT pair buffer (Pool convert) for DoubleRow pv matmuls
            even = jt % 2 == 0
            if even:
                p8 = pt8p.tile([C, 1024], FP8, tag="p8", name="p8")
                p8_live[0] = p8
            else:
                p8 = p8_live[0]
            nc.gpsimd.tensor_copy(p8[:, 0 if even else 512:512 if even else 1024], ptt[:])
            for t in range(4):
                tsl = bass.ts(t, 128)
                # den: exact bf16 P against ones (zeroes the amsden bank once)
                nc.tensor.matmul(amsden[:, 2 * t + 1:2 * t + 2], pb[:, tsl], ones_bcol[:],
                                 start=(first and t == 0), stop=last, skip_group_check=True)
            if not even:
                p8r = p8[:].rearrange("p (two i) -> p two i", two=2)
                augr = v_s_aug[:, bass.ts(jt // 2, 256)].rearrange("p (two c) -> p two c", two=2)
                for t in range(4):
                    tsl = slice(t * 128, (t + 1) * 128)
                    nc.tensor.matmul(pv[:, bass.ts(t, 128)], p8r[:, :, tsl], augr,
                                     start=(jt == 1 and t == 0), stop=last,
                                     perf_mode=mybir.MatmulPerfMode.DoubleRow,
                                     skip_group_check=True)
                    nc.tensor.matmul(amsden[:, 2 * t + 0:2 * t + 1], p8r[:, :, tsl],
                                     ones8[:].rearrange("p (two o) -> p two o", o=1),
                                     start=False, stop=last,
                                     perf_mode=mybir.MatmulPerfMode.DoubleRow,
                                     skip_group_check=True)

        def emit_epilogue_tile(ic, t, pv, amsden):
            it = ic * 4 + t
            acol = amsden[:, 2 * t + 0:2 * t + 1]
            dcol = amsden[:, 2 * t + 1:2 * t + 2]
            rcol = stmp.tile([C, 1], F32, tag="rcol", name="rcol")
            nc.vector.reciprocal_approx_fast(out=rcol[:], in_=dcol)
            x1 = stmp.tile([C, C], F32, tag="x1", name="x1")
            nc.vector.scalar_tensor_tensor(
                out=x1[:], in0=v_r_nat[:, bass.ts(it, 128)], scalar=acol,
                in1=pv[:, bass.ts(t, 128)], op0=ALU.mult, op1=ALU.add)
            x2 = stmp.tile([C, C], BF16, tag="x2", name="x2")
            nc.vector.tensor_scalar(
                out=x2[:], in0=x1[:], scalar1=rcol[:, 0:1], scalar2=1.0 / AEXP,
                op0=ALU.mult, op1=ALU.mult)
            pso = ps_tp.tile([C, C], BF16, tag="mm", name="tp")
            nc.tensor.transpose(pso[:], x2[:], idb[:])
            nc.vector.tensor_copy(outT_pre[:, bass.ts(it, 128)], pso[:])

        def emit_epilogue_proj(ic):
            isl = bass.ts(ic, 512)
            pspj = ps_mm.tile([C, 512], F32, tag="mm", name="mm")
            nc.tensor.matmul(pspj[:], wprojb[:], outT_pre[:, isl], start=True, stop=True)
            obj = stmp.tile([C, 512], BF16, tag="ob", name="ob")
            nc.scalar.activation(obj[:], pspj[:], AF.Identity, bias=bp)
            nc.sync.dma_start(outT_d[:, isl], obj[:])

        # ---- emission ----
        xc0 = stream3(xT_d, 0)
        sc0 = stream3(sendT_d, 0)
        rc0 = stream3(res_sT_d, 0)
        rcv = strm.tile([C, 1536], BF16, tag="instream", name="rcv")
        nc.sync.dma_start(rcv[:], recvTo_d)
        rrc = strm.tile([C, 1536], BF16, tag="instream", name="rrc")
        nc.sync.dma_start(rrc[:], res_rTo_d)
        nc.sync.dma_start(wpack[:, 4 * C:], wpack_d[:, 4 * C:].bitcast(F32R))
        for jt in range(ITI):
            load_mask(jt)
        xc1 = stream3(xT_d, 1)
        sc1 = stream3(sendT_d, 1)
        rc1 = stream3(res_sT_d, 1)
        for jt in range(ITI, JT):
            load_mask(jt)
        # own-half senders, then receivers (each internally stage-pipelined;
        # running both at once oversubscribes the 4-bank mm rotation)
        for _ in sender_half_gen(xc0, sc0, rc0, 0):
            pass
        for _ in receiver_half_gen(xc0, rcv, rrc):
            pass
        for _ in sender_half_gen(xc1, sc1, rc1, 1):
            pass

        pvs, dens = {}, {}
        epi_q = []
        for ic in range(ICH):
            pvs[ic] = ps_pv.tile([C, 512], F32, tag="pv", name="pv")
            dens[ic] = ps_dn.tile([C, 8], F32, tag="den", name="den")
            for jt in range(JT):
                emit_attn(ic, jt, pvs[ic], dens[ic], first=(jt == 0), last=(jt == JT - 1))
                if jt >= 2 and epi_q:
                    eic, et = epi_q.pop(0)
                    if et == "proj":
                        emit_epilogue_proj(eic)
                    else:
                        emit_epilogue_tile(eic, et, pvs[eic], dens[eic])
            epi_q.extend((ic, t) for t in range(4))
            epi_q.append((ic, "proj"))
        for eic, et in epi_q:
            if et == "proj":
                emit_epilogue_proj(eic)
            else:
                emit_epilogue_tile(eic, et, pvs[eic], dens[eic])

    nc.compile()
    return nc


def _host_prep(inputs):
    """Returns list of 8 per-core input dicts."""
    f32 = np.float32
    fp8np = mybir.dt.np(FP8)
    x = np.asarray(inputs["x"], f32)
    recv = np.asarray(inputs["receiver_val_res"], f32)
    res_r = np.asarray(inputs["residual_receiver"], f32)
    send = np.asarray(inputs["sender_val_res"], f32)
    res_s = np.asarray(inputs["residual_sender"], f32)
    mask = np.asarray(inputs["attn_mask"])
    ra = np.asarray(inputs["relation_attn"], f32)
    q_w = np.asarray(inputs["q_w"], f32)
    proj_w = np.asarray(inputs["proj_w"], f32)
    proj_b = np.asarray(inputs["proj_b"], f32)
    r_w = np.asarray(inputs["r_w"], f32)
    r_b = np.asarray(inputs["r_b"], f32)
    s_w = np.asarray(inputs["s_w"], f32)
    s_b = np.asarray(inputs["s_b"], f32)
    n_weight = np.asarray(inputs["n_weight"], f32)
    n_bias = np.asarray(inputs["n_bias"], f32)

    mem_w, recv_w, send_w = ra[:, :C], ra[:, C:2 * C], ra[:, 2 * C:]
    w_proj_eff = proj_w * n_weight[None, :]
    b_proj_eff = proj_w @ n_bias + proj_b

    cc = np.ascontiguousarray
    wpack = np.concatenate([
        send_w.T, mem_w.T, recv_w.T, q_w.T * SCALE, w_proj_eff.T, r_w.T, s_w.T,
        np.eye(C, dtype=f32), np.full((C, C), 1.0 / C, f32),
        b_proj_eff[:, None], r_b[:, None], s_b[:, None],
        np.full((C, 1), 1.0 / C, f32), np.ones((C, 1), f32),
        np.full((C, 1), LNA, f32), np.zeros((C, 1), f32),
    ], axis=1).astype(f32)
    weights = {
        "wpack": cc(wpack),
        "idb": cc(np.eye(C).astype(ml_dtypes.bfloat16)),
        "idb8": cc(np.eye(C).astype(fp8np)),
        "wprojb": cc(w_proj_eff.T.astype(ml_dtypes.bfloat16)),
        "wbf": cc(np.concatenate([send_w.T, mem_w.T, recv_w.T, q_w.T * SCALE],
                                 axis=1).astype(ml_dtypes.bfloat16)),
        "ones_row": np.ones((1, C), f32),
    }

    in_maps = []
    for core in range(8):
        b, half = core // 2, core % 2
        i0, i1 = half * NO, (half + 1) * NO
        # sender-axis permutation: own half first (program assumes own = [0, NO))
        perm = np.concatenate([np.arange(i0, i1), np.arange(0, i0), np.arange(i1, N)])
        xb = x[:, b, :].T                          # [C, N]
        mT = mask[b, 0, i0:i1, :].T.astype(f32)    # [N(j), NO(i)]
        bfd = ml_dtypes.bfloat16
        m = {
            "xT": cc(xb[:, perm].astype(bfd)),
            "sendT": cc(send[:, b, :].T[:, perm].astype(bfd)),
            "res_sT": cc(res_s[:, b, :].T[:, perm].astype(bfd)),
            "recvTo": cc(recv[i0:i1, b, :].T.astype(bfd)),
            "res_rTo": cc(res_r[i0:i1, b, :].T.astype(bfd)),
            "maskT": cc((mT[perm, :] * (-MASKM)).astype(fp8np)),
        }
        m.update(weights)
        in_maps.append(m)
    return in_maps


def kernel(**inputs):
    if "nc" not in _CACHE:
        _CACHE["nc"] = _build_program()
    nc = _CACHE["nc"]
    in_maps = _host_prep(inputs)
    res = run_bass_kernel_spmd(nc, in_maps, core_ids=list(range(8)))
    out = np.zeros((N, B, C), np.float32)
    vr2 = np.zeros((N, B, C), np.float32)
    vs2 = np.zeros((N, B, C), np.float32)
    for core in range(8):
        b, half = core // 2, core % 2
        i0, i1 = half * NO, (half + 1) * NO
        r = res.results[core]
        out[i0:i1, b, :] = np.asarray(r["outT"], np.float32).T
        vr2[i0:i1, b, :] = np.asarray(r["vr2T"], np.float32).T
        vs2[i0:i1, b, :] = np.asarray(r["vs2T"], np.float32).T
    return out, vr2, vs2
